# revision 28
# baseline (speedup 1.0000x reference)
"""Differentiable voxelizer (winding-number occupancy) on 8 Trainium2 cores.

Math: for each grid point p, w(p) = (1/4pi) * sum_n (q_n - p) . na_n / (|q_n - p|^3 + eps)
Factorized as w(p) = (D(p) - p' . M(p)) / 4pi with
    t_pn = 1 / (r_pn^3 + eps)
    D(p) = sum_n ((q_n - o) . na_n) t_pn     (1 row of the reduction matmul)
    M(p) = sum_n na_n t_pn                   (3 rows of the reduction matmul)
where o is a per-block origin (p' = p - o), and r^2 is computed by a K=9
matmul whose rows are ordered as per-axis triplets
    (px'^2, -2 px' qx', qx'^2, py'^2, ...)
so the PE's sequential accumulation keeps partial sums near zero -- together
with the origin shift this makes the fp32 cancellation noise in r^2 ~20x
smaller than the naive 5-term form.

Device kernel (per core, verts on partitions, points on the free axis):
    mm1 (PE, fp32):  r2[vert, pt]           (K=9 matmul, triplet-ordered)
    ACT: r = sqrt(r2)                       (adaptive bias guards negatives)
    DVE: r3 = max(r2, 0) * r                (fused scalar_tensor_tensor)
    ACT: r3e = r3 + EPS_DEV                 (Copy activation with bias)
    DVE: t = reciprocal_approx_fast(r3e)    (written as float32r)
    mm2 (PE, f32r): DM[0:4, pt] += [d, na].T @ t   (PSUM-accumulated)

Accuracy scheme: the device evaluates t with an inflated floor EPS_DEV and
f32r-rounded weights; the host adds, for every pair with r < R_CUT, the exact
correction  numer/(r^3+E_EPS) - numer_dev*t_dev(r)  in float64, where
numer_dev/t_dev replicate exactly what the device used (f32r grid is RTN-11,
hardware-verified). Sharding: data-parallel over grid points (spatial boxes)
across the 8 cores; verts/normals replicated; host gathers the per-core
(4, Npc) DM blocks, applies the final affine + sigmoid, trilinear-resizes and
scatters into the (R, R, R) volume.
"""
import sys
import numpy as np

if '/opt/trn_rl_repo' not in sys.path:
    sys.path.insert(0, '/opt/trn_rl_repo')

BBOX_DENSITY = 64
E_EPS = 1e-6
N_CORES = 8
FD = 1024          # point block size (free dim of the elementwise ops)
MM_N = 512         # max fp32 matmul moving free dim
MM2_F32R = True    # f32r reduction matmul (weights pre-rounded host-side)
EPS_DEV = 1e-5     # device denominator floor (host corrects within R_CUT)
R_CUT = 0.25       # host-corrected pair radius
_FP32_R2_ERR = 4e-7   # bound on device r2 rounding noise (post shift+triplets)

# ---------------------------------------------------------------- host math

def _grid_and_layout(verts_np, R):
    bmin, bmax = verts_np.min(0), verts_np.max(0)
    blen = bmax - bmin
    step_len = blen.max() / BBOX_DENSITY
    step = (blen / step_len).astype(np.int64) + 1
    axes = [np.linspace(bmin[i], bmax[i], step[i]) for i in range(3)]
    gx, gy, gz = np.meshgrid(*axes, indexing='ij')
    coords = np.stack([gx, gy, gz], -1)
    order = np.argsort(step)
    inv = np.argsort(order)
    s = step[order]
    coords = coords.transpose(tuple(order) + (3,)).reshape(s[0], s[1] * s[2], 3).astype(np.float32)
    bidx = np.floor((np.stack([bmin, bmax], 1) + 1.0) * R / 2.0).astype(np.int64)
    return coords, tuple(int(x) for x in s), tuple(int(x) for x in inv), bidx


def _normals_areaic(verts, faces):
    verts = np.asarray(verts, np.float32)
    faces = np.asarray(faces)
    V = verts.shape[0]
    v0, v1, v2 = verts[faces[:, 0]], verts[faces[:, 1]], verts[faces[:, 2]]
    A, B, C = v1 - v0, v2 - v1, v0 - v2
    cross = np.cross(A, v2 - v0).astype(np.float32)
    face_areas = (0.5 * np.linalg.norm(cross, axis=-1)).astype(np.float32)
    vids = faces.T.reshape(-1)
    vn = np.zeros((V, 3), np.float32)
    np.add.at(vn, vids, np.tile(cross, (3, 1)))
    vn = vn / np.maximum(np.linalg.norm(vn, axis=-1, keepdims=True), 1e-12).astype(np.float32)
    nA = np.linalg.norm(A, axis=1).astype(np.float32)
    nB = np.linalg.norm(B, axis=1).astype(np.float32)
    nC = np.linalg.norm(C, axis=1).astype(np.float32)
    a0 = np.arccos(np.clip(-np.sum(A * C, 1) / (1e-10 + nA * nC), -1.0, 1.0))
    a1 = np.arccos(np.clip(-np.sum(A * B, 1) / (1e-10 + nA * nB), -1.0, 1.0))
    a2 = np.arccos(np.clip(-np.sum(B * C, 1) / (1e-10 + nB * nC), -1.0, 1.0))
    angles = np.stack([a0, a1, a2], 1).astype(np.float32)
    s2 = np.sin(2.0 * angles).astype(np.float32)
    w = s2 / (np.sum(s2, -1, keepdims=True) + np.float32(1e-8))
    w = (w[:, [2, 0, 1]] + w[:, [1, 2, 0]]) * np.float32(0.5)
    dual = np.zeros(V, np.float32)
    np.add.at(dual, faces.reshape(-1), (w * face_areas[:, None]).reshape(-1))
    return (vn * dual[:, None]).astype(np.float32)


def _trilerp(vol, out_sizes):
    vol = np.asarray(vol, np.float32)
    cs = [((np.arange(O).astype(np.float32) + np.float32(0.5)) * np.float32(I / O)
           - np.float32(0.5)) for I, O in zip(vol.shape, out_sizes)]
    idx0, frac = [], []
    for c in cs:
        i0 = np.floor(c)
        frac.append((c - i0).astype(np.float32))
        idx0.append(i0.astype(np.int64))
    out = np.zeros(out_sizes, np.float32)
    for dx in (0, 1):
        ix = np.clip(idx0[0] + dx, 0, vol.shape[0] - 1)
        wx = (frac[0] if dx else (np.float32(1.0) - frac[0])).astype(np.float32)
        for dy in (0, 1):
            iy = np.clip(idx0[1] + dy, 0, vol.shape[1] - 1)
            wy = (frac[1] if dy else (np.float32(1.0) - frac[1])).astype(np.float32)
            for dz in (0, 1):
                iz = np.clip(idx0[2] + dz, 0, vol.shape[2] - 1)
                wz = (frac[2] if dz else (np.float32(1.0) - frac[2])).astype(np.float32)
                out += (vol[np.ix_(ix, iy, iz)]
                        * wx[:, None, None] * wy[None, :, None] * wz[None, None, :])
    return out


def _assemble(occ_flat, s, inv, bidx, R):
    vol = occ_flat.reshape(s).transpose(inv)
    sizes = tuple(int(bidx[i, 1] - bidx[i, 0] + 1) for i in range(3))
    box = _trilerp(vol, sizes)
    whole = np.zeros((R, R, R), dtype=box.dtype)
    whole[bidx[0, 0]:bidx[0, 1] + 1, bidx[1, 0]:bidx[1, 1] + 1,
          bidx[2, 0]:bidx[2, 1] + 1] = box
    return whole.transpose(2, 1, 0)[None]


def _sigmoid100(w):
    x = (np.asarray(w, np.float32) - np.float32(0.5)) * np.float32(100.0)
    with np.errstate(over='ignore'):
        return (np.float32(1.0) / (np.float32(1.0) + np.exp(-x))).astype(np.float32)


def _round_f32r(a):
    """Round fp32 values onto the PE's f32r grid (round-to-nearest, 11
    explicit mantissa bits -- hardware-verified for the copy op and both
    matmul operand paths)."""
    a32 = np.ascontiguousarray(a, np.float32)
    bits = a32.view(np.uint32)
    r = ((bits + np.uint32(0x1000)) & np.uint32(0xFFFFE000)).astype(np.uint32)
    return r.view(np.float32).copy()


def _choose_boxes(s, steps):
    """Pick a (bz, by, bx) box shape with bz*by*bx <= FD minimizing box count
    then physical diameter. Returns box dims."""
    best = None
    for bz in range(1, min(s[0], FD) + 1):
        for by in range(1, min(s[1], FD // bz) + 1):
            bx = min(s[2], FD // (bz * by))
            nb = ((s[0] + bz - 1) // bz) * ((s[1] + by - 1) // by) * ((s[2] + bx - 1) // bx)
            ext = ((bz - 1) * steps[0], (by - 1) * steps[1], (bx - 1) * steps[2])
            diam = sum(e * e for e in ext)
            key = (nb, diam)
            if best is None or key < best[0]:
                best = (key, (bz, by, bx))
    return best[1]


def _near_pairs(pts, verts, r_cut):
    """(point_idx, vert_idx) arrays for all pairs with |p-q| <= r_cut, plus
    the min pair distance squared. Uses cKDTree when available."""
    try:
        from scipy.spatial import cKDTree
        tp = cKDTree(pts)
        tv = cKDTree(verts)
        dmin, _ = tv.query(pts, k=1, workers=-1)
        min_r2 = float((dmin ** 2).min())
        coo = tp.sparse_distance_matrix(tv, r_cut, output_type='coo_matrix')
        return coo.row.astype(np.int64), coo.col.astype(np.int64), min_r2
    except Exception:
        pass
    p32 = pts.astype(np.float32)
    v32 = verts.astype(np.float32)
    p2 = (p32 ** 2).sum(1)
    q2 = (v32 ** 2).sum(1)
    rows, cols = [], []
    best = np.inf
    thresh = np.float32(r_cut * r_cut * 1.02 + 1e-5)
    B = 16384
    for i in range(0, pts.shape[0], B):
        r2 = (p2[i:i + B, None] + q2[None, :]) - 2.0 * (p32[i:i + B] @ v32.T)
        best = min(best, float(r2.min()))
        ii, jj = np.nonzero(r2 <= thresh)
        rows.append(ii + i)
        cols.append(jj)
    return np.concatenate(rows), np.concatenate(cols), max(best, 0.0)


def _near_field(pts, verts, na, eps_dev, r_cut, na_dev64, d_dev64, blk_of_pt, o64):
    """Host correction: for pairs with r < r_cut accumulates
    numer/(r^3+E_EPS) - numer_dev*t_dev  (float64, /4pi), where numer_dev and
    t_dev replicate the device's f32r-rounded weights and t. Also returns the
    min pair r^2."""
    n = pts.shape[0]
    ii, jj, min_r2 = _near_pairs(pts, verts, r_cut)
    v64 = verts.astype(np.float64)
    na64 = na.astype(np.float64)
    w_corr = np.zeros(n, np.float64)
    B = 4 << 20
    for k in range(0, ii.size, B):
        i_, j_ = ii[k:k + B], jj[k:k + B]
        pex = pts[i_].astype(np.float64)
        d = v64[j_] - pex
        r2e = (d * d).sum(1)
        keep = r2e <= r_cut * r_cut
        i_, j_, d, r2e, pex = i_[keep], j_[keep], d[keep], r2e[keep], pex[keep]
        r3 = r2e * np.sqrt(r2e)
        numer = (d * na64[j_]).sum(1)
        blk = blk_of_pt[i_]
        pprime = (pex - o64[blk]).astype(np.float32).astype(np.float64)
        numer_dev = d_dev64[blk, j_] - (pprime * na_dev64[j_]).sum(1)
        t_far = 1.0 / (r3 + eps_dev)
        if MM2_F32R:
            t_far = _round_f32r(t_far.astype(np.float32)).astype(np.float64)
        corr = numer / (r3 + E_EPS) - numer_dev * t_far
        w_corr += np.bincount(i_, weights=corr, minlength=n)
    return min_r2, w_corr / (4.0 * np.pi)


# ---------------------------------------------------------------- device kernel

_PROGRAM_CACHE = {}
LAST_RESULTS = None


def _build_program(npc, vp, sqrt_bias=0.0, eps_dev=EPS_DEV):
    """Bass program for one core: npc points (multiple of FD), vp verts
    (multiple of 128). Per-block q9/dna stream from DRAM."""
    import concourse.tile as tile
    from concourse import bacc, mybir
    from concourse.dve_ops import RECIP_APPROX_FAST_CONSTS, RECIPROCAL_APPROX_FAST
    from contextlib import ExitStack

    vt_n = vp // 128
    c_n = npc // FD
    nc = bacc.Bacc("TRN2", target_bir_lowering=False, debug=False,
                   num_devices=N_CORES)
    q9_d = nc.dram_tensor("q9", [9, c_n, vp], mybir.dt.float32, kind="ExternalInput").ap()
    dna_d = nc.dram_tensor("dna", [128, c_n, vt_n, 4], mybir.dt.float32, kind="ExternalInput").ap()
    p9_d = nc.dram_tensor("p9", [9, npc], mybir.dt.float32, kind="ExternalInput").ap()
    out_d = nc.dram_tensor("dm", [4, npc], mybir.dt.float32, kind="ExternalOutput").ap()

    act, alu = mybir.ActivationFunctionType, mybir.AluOpType
    mm2_dt = mybir.dt.float32r if MM2_F32R else mybir.dt.float32

    with tile.TileContext(nc) as tc:
        with ExitStack() as ctx:
            const = ctx.enter_context(tc.tile_pool(name="const", bufs=1))
            blkio = ctx.enter_context(tc.tile_pool(name="blkio", bufs=2))
            work = ctx.enter_context(tc.tile_pool(name="work", bufs=3))
            psr2 = ctx.enter_context(tc.tile_pool(name="psr2", bufs=2, space="PSUM"))
            psdm = ctx.enter_context(tc.tile_pool(name="psdm", bufs=2, space="PSUM"))

            if sqrt_bias != 0.0:
                bias_t = const.tile([128, 1], mybir.dt.float32)
                nc.vector.memset(bias_t[:], sqrt_bias)
                bias_ap = bias_t[:]
            else:
                bias_ap = 0.0
            p9 = const.tile([9, npc], mybir.dt.float32)
            nc.sync.dma_start(p9[:], p9_d[:])

            for c in range(c_n):
                q9c = blkio.tile([9, vp], mybir.dt.float32, tag="q9c")
                nc.sync.dma_start(q9c[:], q9_d[:, c, :])
                dnac = blkio.tile([128, vt_n, 4], mybir.dt.float32, tag="dnac")
                nc.sync.dma_start(dnac[:], dna_d[:, c, :, :])
                if MM2_F32R:
                    dna_r = blkio.tile([128, vt_n, 4], mybir.dt.float32r, tag="dnar")
                    nc.vector.tensor_copy(dna_r[:], dnac[:])
                else:
                    dna_r = dnac
                dm = psdm.tile([4, FD], mybir.dt.float32)
                for vt in range(vt_n):
                    r2 = psr2.tile([128, FD], mybir.dt.float32)
                    for h in range(FD // MM_N):
                        nc.tensor.matmul(
                            r2[:, h * MM_N:(h + 1) * MM_N],
                            q9c[:, vt * 128:(vt + 1) * 128],
                            p9[:, c * FD + h * MM_N: c * FD + (h + 1) * MM_N],
                            start=True, stop=True)
                    r = work.tile([128, FD], mybir.dt.float32, tag="r")
                    nc.scalar.activation(r[:], r2[:], act.Sqrt, bias=bias_ap)
                    r3 = work.tile([128, FD], mybir.dt.float32, tag="r3")
                    nc.vector.scalar_tensor_tensor(
                        out=r3[:], in0=r2[:], scalar=0.0, in1=r[:],
                        op0=alu.max, op1=alu.mult)
                    r3e = work.tile([128, FD], mybir.dt.float32, tag="r3e")
                    nc.scalar.activation(r3e[:], r3[:], act.Copy, bias=float(eps_dev))
                    t = work.tile([128, FD], mm2_dt, tag="t")
                    if MM2_F32R:
                        cst = RECIP_APPROX_FAST_CONSTS
                        nc.vector._custom_dve(
                            RECIPROCAL_APPROX_FAST, out=t[:], in0=r3e[:],
                            s0=cst["s0"], s1=cst["s1"], imm2=cst["imm2"])
                    else:
                        nc.vector.reciprocal_approx_fast(out=t[:], in_=r3e[:])
                    for h in range(FD // MM_N):
                        nc.tensor.matmul(
                            dm[:, h * MM_N:(h + 1) * MM_N],
                            dna_r[:, vt, :],
                            t[:, h * MM_N:(h + 1) * MM_N],
                            start=(vt == 0), stop=(vt == vt_n - 1))
                dm_sb = work.tile([4, FD], mybir.dt.float32, tag="dm_sb")
                nc.scalar.copy(dm_sb[:], dm[:])
                nc.sync.dma_start(out_d[:, c * FD:(c + 1) * FD], dm_sb[:])
    nc.compile()
    return nc


def _winding_device(pts, verts, na, s):
    """pts (N,3) float32 in grid order for grid shape s -> w (N,) float64."""
    from concourse.bass_utils import run_bass_kernel_spmd
    global LAST_RESULTS

    n = pts.shape[0]
    v = verts.shape[0]
    vp = ((v + 127) // 128) * 128
    assert n == s[0] * s[1] * s[2]

    # spatial boxes -> blocks of FD slots
    bmin = pts.min(0).astype(np.float64)
    bmax = pts.max(0).astype(np.float64)
    steps = [(bmax[i] - bmin[i]) / max(s[i] - 1, 1) for i in range(3)]
    # pts grid order: axis0 slowest; physical steps per axis
    bz, by, bx = _choose_boxes(s, steps)
    nbz = (s[0] + bz - 1) // bz
    nby = (s[1] + by - 1) // by
    nbx = (s[2] + bx - 1) // bx
    n_blocks = nbz * nby * nbx
    c_n = (n_blocks + N_CORES - 1) // N_CORES
    npc = c_n * FD
    ntot = npc * N_CORES
    total_blocks = c_n * N_CORES

    gidx = np.arange(n).reshape(s)
    slot_orig = np.full((total_blocks, FD), -1, np.int64)
    b = 0
    for iz in range(nbz):
        for iy in range(nby):
            for ix in range(nbx):
                ids = gidx[iz * bz:(iz + 1) * bz,
                           iy * by:(iy + 1) * by,
                           ix * bx:(ix + 1) * bx].reshape(-1)
                slot_orig[b, :ids.size] = ids
                b += 1
    slot_orig = slot_orig.reshape(-1)          # (ntot,)
    valid = slot_orig >= 0
    blk_of_slot = np.arange(ntot) // FD
    blk_of_pt = np.empty(n, np.int64)
    blk_of_pt[slot_orig[valid]] = blk_of_slot[valid]

    p64 = np.zeros((ntot, 3), np.float64)
    p64[valid] = pts.astype(np.float64)[slot_orig[valid]]

    # per-block origins (fp32 grid-representable)
    o32 = np.zeros((total_blocks, 3), np.float32)
    cnt = np.bincount(blk_of_slot[valid], minlength=total_blocks).astype(np.float64)
    for a in range(3):
        ssum = np.bincount(blk_of_slot[valid], weights=p64[valid, a], minlength=total_blocks)
        o32[:, a] = (ssum / np.maximum(cnt, 1.0)).astype(np.float32)
    o64 = o32.astype(np.float64)

    pp64 = p64 - o64[blk_of_slot]              # p' per slot (pads: -o, harmless)
    pp64[~valid] = 0.0
    pp32 = pp64.astype(np.float32)
    pp64s = pp32.astype(np.float64)            # fp32-rounded p' (device value)
    p9 = np.zeros((9, ntot), np.float32)
    for a in range(3):
        p9[3 * a + 0] = (pp64s[:, a] ** 2).astype(np.float32)
        p9[3 * a + 1] = (-2.0 * pp64s[:, a]).astype(np.float32)
        p9[3 * a + 2] = 1.0

    q64 = np.zeros((vp, 3), np.float64)
    q64[:v] = verts.astype(np.float64)
    na64 = np.zeros((vp, 3), np.float64)
    na64[:v] = na.astype(np.float64)

    # per-block q9 rows [1, qx', qx'^2, 1, qy', qy'^2, 1, qz', qz'^2]
    qp32 = (q64[None, :, :] - o64[:, None, :]).astype(np.float32)  # (B, vp, 3)
    qp64 = qp32.astype(np.float64)
    q9 = np.empty((total_blocks, 9, vp), np.float32)
    for a in range(3):
        q9[:, 3 * a + 0, :] = 1.0
        q9[:, 3 * a + 1, :] = qp32[:, :, a]
        q9[:, 3 * a + 2, :] = (qp64[:, :, a] ** 2).astype(np.float32)

    na_dev = na64.astype(np.float32)
    dvals = np.einsum('bvc,vc->bv', qp64, na64).astype(np.float32)  # (B, vp)
    if MM2_F32R:
        na_dev = _round_f32r(na_dev)
        dvals = _round_f32r(dvals)
    dna = np.zeros((total_blocks, vp // 128, 128, 4), np.float32)
    dna[:, :, :, 0] = dvals.reshape(total_blocks, vp // 128, 128)
    dna[:, :, :, 1:4] = np.broadcast_to(
        na_dev.reshape(1, vp // 128, 128, 3), (total_blocks, vp // 128, 128, 3))
    dna = np.ascontiguousarray(dna.transpose(2, 0, 1, 3))  # (128, B, vt_n, 4)

    # near-field host correction + min pair distance (one blocked scan)
    min_r2, w_corr = _near_field(
        pts, verts, na, EPS_DEV, R_CUT,
        na_dev[:v].astype(np.float64), dvals[:, :v].astype(np.float64),
        blk_of_pt, o64)
    sqrt_bias = 0.0 if min_r2 >= 4.0 * _FP32_R2_ERR else float(2.0 * _FP32_R2_ERR)

    key = (npc, vp, sqrt_bias, MM2_F32R, EPS_DEV)
    if key not in _PROGRAM_CACHE:
        _PROGRAM_CACHE[key] = _build_program(npc, vp, sqrt_bias, EPS_DEV)
    nc = _PROGRAM_CACHE[key]

    in_maps = []
    for core in range(N_CORES):
        bsl = slice(core * c_n, (core + 1) * c_n)
        in_maps.append({
            "q9": np.ascontiguousarray(q9[bsl].transpose(1, 0, 2)),
            "dna": np.ascontiguousarray(dna[:, bsl]),
            "p9": np.ascontiguousarray(p9[:, core * npc:(core + 1) * npc]),
        })
    res = run_bass_kernel_spmd(nc, in_maps, core_ids=list(range(N_CORES)))
    LAST_RESULTS = res

    dm = np.concatenate([r["dm"] for r in res.results], axis=1)  # (4, ntot)
    dm64 = dm.astype(np.float64)
    w_slot = dm64[0] - (pp64s[:, 0] * dm64[1] + pp64s[:, 1] * dm64[2]
                        + pp64s[:, 2] * dm64[3])
    w = np.empty(n, np.float64)
    w[slot_orig[valid]] = w_slot[valid]
    return w / (4.0 * np.pi) + w_corr


def kernel(verts, faces, output_resolution):
    verts = np.asarray(verts, np.float32)
    faces = np.asarray(faces)
    R = int(output_resolution)
    coords, s, inv, bidx = _grid_and_layout(verts, R)
    na = _normals_areaic(verts, faces)
    pts = coords.reshape(-1, 3)
    w = _winding_device(pts, verts, na, s)
    occ = _sigmoid100(w)
    return _assemble(occ, s, inv, bidx, R)


# revision 36
# speedup vs baseline: 1.1229x; 1.1229x over previous
"""Differentiable voxelizer (winding-number occupancy) on 8 Trainium2 cores.

Math: for each grid point p, w(p) = (1/4pi) * sum_n (q_n - p) . na_n / (|q_n - p|^3 + eps)
Factorized as w(p) = (D(p) - p' . M(p)) / 4pi with
    t_pn = 1 / (r_pn^3 + eps)
    D(p) = sum_n ((q_n - o) . na_n) t_pn     (1 row of the reduction matmul)
    M(p) = sum_n na_n t_pn                   (3 rows of the reduction matmul)
where o is a per-block origin (p' = p - o), and r^2 is computed by a K=9
matmul whose rows are ordered as per-axis triplets
    (px'^2, -2 px' qx', qx'^2, py'^2, ...)
so the PE's sequential accumulation keeps partial sums near zero -- together
with the origin shift this makes the fp32 cancellation noise in r^2 ~20x
smaller than the naive 5-term form.

Device kernel (per core, verts on partitions, points on the free axis):
    mm1 (PE, fp32):  r2[vert, pt]           (K=9 matmul, triplet-ordered)
    ACT: r = sqrt(r2)                       (adaptive bias guards negatives)
    DVE: r3 = max(r2, 0) * r                (fused scalar_tensor_tensor)
    ACT: r3e = r3 + EPS_DEV                 (Copy activation with bias)
    DVE: t = reciprocal_approx_fast(r3e)    (written as float32r)
    mm2 (PE, f32r): DM[0:4, pt] += [d, na].T @ t   (PSUM-accumulated)

Accuracy scheme: the device evaluates t with an inflated floor EPS_DEV and
f32r-rounded weights; the host adds, for every pair with r < R_CUT, the exact
correction  numer/(r^3+E_EPS) - numer_dev*t_dev(r)  in float64, where
numer_dev/t_dev replicate exactly what the device used (f32r grid is RTN-11,
hardware-verified). Sharding: data-parallel over grid points (spatial boxes)
across the 8 cores; verts/normals replicated; host gathers the per-core
(4, Npc) DM blocks, applies the final affine + sigmoid, trilinear-resizes and
scatters into the (R, R, R) volume.
"""
import sys
import numpy as np

if '/opt/trn_rl_repo' not in sys.path:
    sys.path.insert(0, '/opt/trn_rl_repo')

BBOX_DENSITY = 64
E_EPS = 1e-6
N_CORES = 8
FD = 1024          # point block size (free dim of the elementwise ops)
MM_N = 512         # max fp32 matmul moving free dim
MM2_F32R = True    # f32r reduction matmul (weights pre-rounded host-side)
MM1_F32R = True    # f32r r^2 matmul: rows pre-rounded to the f32r grid and the
                   # host correction replicates r^2 as the exact sum of rounded
                   # products, so quantization cancels for corrected pairs
EPS_DEV = 1e-5     # device denominator floor (host corrects within R_CUT)
R_CUT = 0.25       # host-corrected pair radius
_FP32_R2_ERR = 4e-7   # bound on device r2 accumulation noise (shift+triplets)
# f32r square-row defects can push computed r2 ~ -5e-5; the bias is replicated
# by the host correction, so it costs no accuracy on corrected pairs
_MM1_F32R_SQRT_BIAS = 8e-5

# ---------------------------------------------------------------- host math

def _grid_and_layout(verts_np, R):
    bmin, bmax = verts_np.min(0), verts_np.max(0)
    blen = bmax - bmin
    step_len = blen.max() / BBOX_DENSITY
    step = (blen / step_len).astype(np.int64) + 1
    axes = [np.linspace(bmin[i], bmax[i], step[i]) for i in range(3)]
    gx, gy, gz = np.meshgrid(*axes, indexing='ij')
    coords = np.stack([gx, gy, gz], -1)
    order = np.argsort(step)
    inv = np.argsort(order)
    s = step[order]
    coords = coords.transpose(tuple(order) + (3,)).reshape(s[0], s[1] * s[2], 3).astype(np.float32)
    bidx = np.floor((np.stack([bmin, bmax], 1) + 1.0) * R / 2.0).astype(np.int64)
    return coords, tuple(int(x) for x in s), tuple(int(x) for x in inv), bidx


def _normals_areaic(verts, faces):
    verts = np.asarray(verts, np.float32)
    faces = np.asarray(faces)
    V = verts.shape[0]
    v0, v1, v2 = verts[faces[:, 0]], verts[faces[:, 1]], verts[faces[:, 2]]
    A, B, C = v1 - v0, v2 - v1, v0 - v2
    cross = np.cross(A, v2 - v0).astype(np.float32)
    face_areas = (0.5 * np.linalg.norm(cross, axis=-1)).astype(np.float32)
    vids = faces.T.reshape(-1)
    vn = np.zeros((V, 3), np.float32)
    np.add.at(vn, vids, np.tile(cross, (3, 1)))
    vn = vn / np.maximum(np.linalg.norm(vn, axis=-1, keepdims=True), 1e-12).astype(np.float32)
    nA = np.linalg.norm(A, axis=1).astype(np.float32)
    nB = np.linalg.norm(B, axis=1).astype(np.float32)
    nC = np.linalg.norm(C, axis=1).astype(np.float32)
    a0 = np.arccos(np.clip(-np.sum(A * C, 1) / (1e-10 + nA * nC), -1.0, 1.0))
    a1 = np.arccos(np.clip(-np.sum(A * B, 1) / (1e-10 + nA * nB), -1.0, 1.0))
    a2 = np.arccos(np.clip(-np.sum(B * C, 1) / (1e-10 + nB * nC), -1.0, 1.0))
    angles = np.stack([a0, a1, a2], 1).astype(np.float32)
    s2 = np.sin(2.0 * angles).astype(np.float32)
    w = s2 / (np.sum(s2, -1, keepdims=True) + np.float32(1e-8))
    w = (w[:, [2, 0, 1]] + w[:, [1, 2, 0]]) * np.float32(0.5)
    dual = np.zeros(V, np.float32)
    np.add.at(dual, faces.reshape(-1), (w * face_areas[:, None]).reshape(-1))
    return (vn * dual[:, None]).astype(np.float32)


def _trilerp(vol, out_sizes):
    vol = np.asarray(vol, np.float32)
    cs = [((np.arange(O).astype(np.float32) + np.float32(0.5)) * np.float32(I / O)
           - np.float32(0.5)) for I, O in zip(vol.shape, out_sizes)]
    idx0, frac = [], []
    for c in cs:
        i0 = np.floor(c)
        frac.append((c - i0).astype(np.float32))
        idx0.append(i0.astype(np.int64))
    out = np.zeros(out_sizes, np.float32)
    for dx in (0, 1):
        ix = np.clip(idx0[0] + dx, 0, vol.shape[0] - 1)
        wx = (frac[0] if dx else (np.float32(1.0) - frac[0])).astype(np.float32)
        for dy in (0, 1):
            iy = np.clip(idx0[1] + dy, 0, vol.shape[1] - 1)
            wy = (frac[1] if dy else (np.float32(1.0) - frac[1])).astype(np.float32)
            for dz in (0, 1):
                iz = np.clip(idx0[2] + dz, 0, vol.shape[2] - 1)
                wz = (frac[2] if dz else (np.float32(1.0) - frac[2])).astype(np.float32)
                out += (vol[np.ix_(ix, iy, iz)]
                        * wx[:, None, None] * wy[None, :, None] * wz[None, None, :])
    return out


def _assemble(occ_flat, s, inv, bidx, R):
    vol = occ_flat.reshape(s).transpose(inv)
    sizes = tuple(int(bidx[i, 1] - bidx[i, 0] + 1) for i in range(3))
    box = _trilerp(vol, sizes)
    whole = np.zeros((R, R, R), dtype=box.dtype)
    whole[bidx[0, 0]:bidx[0, 1] + 1, bidx[1, 0]:bidx[1, 1] + 1,
          bidx[2, 0]:bidx[2, 1] + 1] = box
    return whole.transpose(2, 1, 0)[None]


def _sigmoid100(w):
    x = (np.asarray(w, np.float32) - np.float32(0.5)) * np.float32(100.0)
    with np.errstate(over='ignore'):
        return (np.float32(1.0) / (np.float32(1.0) + np.exp(-x))).astype(np.float32)


def _round_f32r(a):
    """Round fp32 values onto the PE's f32r grid (round-to-nearest, 11
    explicit mantissa bits -- hardware-verified for the copy op and both
    matmul operand paths)."""
    a32 = np.ascontiguousarray(a, np.float32)
    bits = a32.view(np.uint32)
    r = ((bits + np.uint32(0x1000)) & np.uint32(0xFFFFE000)).astype(np.uint32)
    return r.view(np.float32).copy()


def _choose_boxes(s, steps):
    """Pick a (bz, by, bx) box shape with bz*by*bx <= FD minimizing box count
    then physical diameter. Returns box dims."""
    best = None
    for bz in range(1, min(s[0], FD) + 1):
        for by in range(1, min(s[1], FD // bz) + 1):
            bx = min(s[2], FD // (bz * by))
            nb = ((s[0] + bz - 1) // bz) * ((s[1] + by - 1) // by) * ((s[2] + bx - 1) // bx)
            ext = ((bz - 1) * steps[0], (by - 1) * steps[1], (bx - 1) * steps[2])
            diam = sum(e * e for e in ext)
            key = (nb, diam)
            if best is None or key < best[0]:
                best = (key, (bz, by, bx))
    return best[1]


def _near_pairs(pts, verts, r_cut):
    """(point_idx, vert_idx) arrays for all pairs with |p-q| <= r_cut, plus
    the min pair distance squared. Uses cKDTree when available."""
    try:
        from scipy.spatial import cKDTree
        tp = cKDTree(pts)
        tv = cKDTree(verts)
        dmin, _ = tv.query(pts, k=1, workers=-1)
        min_r2 = float((dmin ** 2).min())
        coo = tp.sparse_distance_matrix(tv, r_cut, output_type='coo_matrix')
        return coo.row.astype(np.int64), coo.col.astype(np.int64), min_r2
    except Exception:
        pass
    p32 = pts.astype(np.float32)
    v32 = verts.astype(np.float32)
    p2 = (p32 ** 2).sum(1)
    q2 = (v32 ** 2).sum(1)
    rows, cols = [], []
    best = np.inf
    thresh = np.float32(r_cut * r_cut * 1.02 + 1e-5)
    B = 16384
    for i in range(0, pts.shape[0], B):
        r2 = (p2[i:i + B, None] + q2[None, :]) - 2.0 * (p32[i:i + B] @ v32.T)
        best = min(best, float(r2.min()))
        ii, jj = np.nonzero(r2 <= thresh)
        rows.append(ii + i)
        cols.append(jj)
    return np.concatenate(rows), np.concatenate(cols), max(best, 0.0)


def _near_field(pts, verts, na, eps_dev, r_cut, na_dev64, d_dev64, blk_of_pt,
                o64, p9r=None, q9r=None, slot_of_pt=None, sqrt_bias=0.0):
    """Host correction: for pairs with r < r_cut accumulates
    numer/(r^3+E_EPS) - numer_dev*t_dev  (float64, /4pi), where numer_dev and
    t_dev replicate the device's rounded weights and t. When MM1_F32R, r^2 for
    t_dev is replicated as the exact sum of the device's rounded row products
    (p9r (9,ntot) x q9r (B,9,vp)). Also returns the min pair r^2."""
    n = pts.shape[0]
    ii, jj, min_r2 = _near_pairs(pts, verts, r_cut)
    v64 = verts.astype(np.float64)
    na64 = na.astype(np.float64)
    w_corr = np.zeros(n, np.float64)
    B = 4 << 20
    for k in range(0, ii.size, B):
        i_, j_ = ii[k:k + B], jj[k:k + B]
        pex = pts[i_].astype(np.float64)
        d = v64[j_] - pex
        r2e = (d * d).sum(1)
        keep = r2e <= r_cut * r_cut
        i_, j_, d, r2e, pex = i_[keep], j_[keep], d[keep], r2e[keep], pex[keep]
        r3 = r2e * np.sqrt(r2e)
        numer = (d * na64[j_]).sum(1)
        blk = blk_of_pt[i_]
        pr32 = (pex - o64[blk]).astype(np.float32)
        if MM1_F32R:
            pr32 = _round_f32r(pr32)
        pprime = pr32.astype(np.float64)
        numer_dev = d_dev64[blk, j_] - (pprime * na_dev64[j_]).sum(1)
        if MM1_F32R:
            # replicate the device r2 = sum of 9 rounded-row products
            prow = p9r[:, slot_of_pt[i_]].astype(np.float64)      # (9, M)
            qrow = q9r[blk, :, j_].astype(np.float64)             # (M, 9)
            r2_rep = np.einsum('km,mk->m', prow, qrow)
            r_rep = np.sqrt(r2_rep + sqrt_bias)
            r3e_rep = np.maximum(r2_rep, 0.0) * r_rep + eps_dev
            t_far = 1.0 / r3e_rep
        else:
            t_far = 1.0 / (r3 + eps_dev)
        if MM2_F32R:
            t_far = _round_f32r(t_far.astype(np.float32)).astype(np.float64)
        corr = numer / (r3 + E_EPS) - numer_dev * t_far
        w_corr += np.bincount(i_, weights=corr, minlength=n)
    return min_r2, w_corr / (4.0 * np.pi)


# ---------------------------------------------------------------- device kernel

_PROGRAM_CACHE = {}
LAST_RESULTS = None


def _build_program(npc, vp, sqrt_bias=0.0, eps_dev=EPS_DEV):
    """Bass program for one core: npc points (multiple of FD), vp verts
    (multiple of 128). Per-block q9/dna stream from DRAM."""
    import concourse.tile as tile
    from concourse import bacc, mybir
    from concourse.dve_ops import RECIP_APPROX_FAST_CONSTS, RECIPROCAL_APPROX_FAST
    from contextlib import ExitStack

    vt_n = vp // 128
    c_n = npc // FD
    nc = bacc.Bacc("TRN2", target_bir_lowering=False, debug=False,
                   num_devices=N_CORES)
    mm1_dt = mybir.dt.float32r if MM1_F32R else mybir.dt.float32
    q9_d = nc.dram_tensor("q9", [9, c_n, vp], mm1_dt, kind="ExternalInput").ap()
    dna_d = nc.dram_tensor("dna", [128, c_n, vt_n, 4], mybir.dt.float32, kind="ExternalInput").ap()
    p9_d = nc.dram_tensor("p9", [9, npc], mm1_dt, kind="ExternalInput").ap()
    out_d = nc.dram_tensor("dm", [4, npc], mybir.dt.float32, kind="ExternalOutput").ap()

    act, alu = mybir.ActivationFunctionType, mybir.AluOpType
    mm2_dt = mybir.dt.float32r if MM2_F32R else mybir.dt.float32

    with tile.TileContext(nc) as tc:
        with ExitStack() as ctx:
            const = ctx.enter_context(tc.tile_pool(name="const", bufs=1))
            blkio = ctx.enter_context(tc.tile_pool(name="blkio", bufs=2))
            work = ctx.enter_context(tc.tile_pool(name="work", bufs=3))
            psr2 = ctx.enter_context(tc.tile_pool(name="psr2", bufs=2, space="PSUM"))
            psdm = ctx.enter_context(tc.tile_pool(name="psdm", bufs=2, space="PSUM"))

            if sqrt_bias != 0.0:
                bias_t = const.tile([128, 1], mybir.dt.float32)
                nc.vector.memset(bias_t[:], sqrt_bias)
                bias_ap = bias_t[:]
            else:
                bias_ap = 0.0
            p9 = const.tile([9, npc], mm1_dt)
            nc.sync.dma_start(p9[:], p9_d[:])

            for c in range(c_n):
                q9c = blkio.tile([9, vp], mm1_dt, tag="q9c")
                nc.sync.dma_start(q9c[:], q9_d[:, c, :])
                dnac = blkio.tile([128, vt_n, 4], mybir.dt.float32, tag="dnac")
                nc.sync.dma_start(dnac[:], dna_d[:, c, :, :])
                if MM2_F32R:
                    dna_r = blkio.tile([128, vt_n, 4], mybir.dt.float32r, tag="dnar")
                    nc.vector.tensor_copy(dna_r[:], dnac[:])
                else:
                    dna_r = dnac
                dm = psdm.tile([4, FD], mybir.dt.float32)
                for vt in range(vt_n):
                    r2 = psr2.tile([128, FD], mybir.dt.float32)
                    for h in range(FD // MM_N):
                        nc.tensor.matmul(
                            r2[:, h * MM_N:(h + 1) * MM_N],
                            q9c[:, vt * 128:(vt + 1) * 128],
                            p9[:, c * FD + h * MM_N: c * FD + (h + 1) * MM_N],
                            start=True, stop=True)
                    r = work.tile([128, FD], mybir.dt.float32, tag="r")
                    nc.scalar.activation(r[:], r2[:], act.Sqrt, bias=bias_ap)
                    r3 = work.tile([128, FD], mybir.dt.float32, tag="r3")
                    nc.vector.scalar_tensor_tensor(
                        out=r3[:], in0=r2[:], scalar=0.0, in1=r[:],
                        op0=alu.max, op1=alu.mult)
                    r3e = work.tile([128, FD], mybir.dt.float32, tag="r3e")
                    nc.scalar.activation(r3e[:], r3[:], act.Copy, bias=float(eps_dev))
                    t = work.tile([128, FD], mm2_dt, tag="t")
                    if MM2_F32R:
                        cst = RECIP_APPROX_FAST_CONSTS
                        nc.vector._custom_dve(
                            RECIPROCAL_APPROX_FAST, out=t[:], in0=r3e[:],
                            s0=cst["s0"], s1=cst["s1"], imm2=cst["imm2"])
                    else:
                        nc.vector.reciprocal_approx_fast(out=t[:], in_=r3e[:])
                    for h in range(FD // MM_N):
                        nc.tensor.matmul(
                            dm[:, h * MM_N:(h + 1) * MM_N],
                            dna_r[:, vt, :],
                            t[:, h * MM_N:(h + 1) * MM_N],
                            start=(vt == 0), stop=(vt == vt_n - 1))
                dm_sb = work.tile([4, FD], mybir.dt.float32, tag="dm_sb")
                nc.scalar.copy(dm_sb[:], dm[:])
                nc.sync.dma_start(out_d[:, c * FD:(c + 1) * FD], dm_sb[:])
    nc.compile()
    return nc


def _winding_device(pts, verts, na, s):
    """pts (N,3) float32 in grid order for grid shape s -> w (N,) float64."""
    from concourse.bass_utils import run_bass_kernel_spmd
    global LAST_RESULTS

    n = pts.shape[0]
    v = verts.shape[0]
    vp = ((v + 127) // 128) * 128
    assert n == s[0] * s[1] * s[2]

    # spatial boxes -> blocks of FD slots
    bmin = pts.min(0).astype(np.float64)
    bmax = pts.max(0).astype(np.float64)
    steps = [(bmax[i] - bmin[i]) / max(s[i] - 1, 1) for i in range(3)]
    # pts grid order: axis0 slowest; physical steps per axis
    bz, by, bx = _choose_boxes(s, steps)
    nbz = (s[0] + bz - 1) // bz
    nby = (s[1] + by - 1) // by
    nbx = (s[2] + bx - 1) // bx
    n_blocks = nbz * nby * nbx
    c_n = (n_blocks + N_CORES - 1) // N_CORES
    npc = c_n * FD
    ntot = npc * N_CORES
    total_blocks = c_n * N_CORES

    gidx = np.arange(n).reshape(s)
    slot_orig = np.full((total_blocks, FD), -1, np.int64)
    b = 0
    for iz in range(nbz):
        for iy in range(nby):
            for ix in range(nbx):
                ids = gidx[iz * bz:(iz + 1) * bz,
                           iy * by:(iy + 1) * by,
                           ix * bx:(ix + 1) * bx].reshape(-1)
                slot_orig[b, :ids.size] = ids
                b += 1
    slot_orig = slot_orig.reshape(-1)          # (ntot,)
    valid = slot_orig >= 0
    blk_of_slot = np.arange(ntot) // FD
    blk_of_pt = np.empty(n, np.int64)
    blk_of_pt[slot_orig[valid]] = blk_of_slot[valid]

    p64 = np.zeros((ntot, 3), np.float64)
    p64[valid] = pts.astype(np.float64)[slot_orig[valid]]

    # per-block origins (fp32 grid-representable)
    o32 = np.zeros((total_blocks, 3), np.float32)
    cnt = np.bincount(blk_of_slot[valid], minlength=total_blocks).astype(np.float64)
    for a in range(3):
        ssum = np.bincount(blk_of_slot[valid], weights=p64[valid, a], minlength=total_blocks)
        o32[:, a] = (ssum / np.maximum(cnt, 1.0)).astype(np.float32)
    o64 = o32.astype(np.float64)

    pp64 = p64 - o64[blk_of_slot]              # p' per slot (pads: -o, harmless)
    pp64[~valid] = 0.0
    pp32 = pp64.astype(np.float32)
    if MM1_F32R:
        pp32 = _round_f32r(pp32)
    pp64s = pp32.astype(np.float64)            # rounded p' (device value)
    p9 = np.zeros((9, ntot), np.float32)
    for a in range(3):
        sq = (pp64s[:, a] ** 2).astype(np.float32)
        li = (-2.0 * pp64s[:, a]).astype(np.float32)
        if MM1_F32R:
            sq = _round_f32r(sq)
            li = _round_f32r(li)
        p9[3 * a + 0] = sq
        p9[3 * a + 1] = li
        p9[3 * a + 2] = 1.0

    q64 = np.zeros((vp, 3), np.float64)
    q64[:v] = verts.astype(np.float64)
    na64 = np.zeros((vp, 3), np.float64)
    na64[:v] = na.astype(np.float64)

    # per-block q9 rows [1, qx', qx'^2, 1, qy', qy'^2, 1, qz', qz'^2]
    qp32 = (q64[None, :, :] - o64[:, None, :]).astype(np.float32)  # (B, vp, 3)
    if MM1_F32R:
        qp32 = _round_f32r(qp32)
    qp64 = qp32.astype(np.float64)
    q9 = np.empty((total_blocks, 9, vp), np.float32)
    for a in range(3):
        sq = (qp64[:, :, a] ** 2).astype(np.float32)
        if MM1_F32R:
            sq = _round_f32r(sq)
        q9[:, 3 * a + 0, :] = 1.0
        q9[:, 3 * a + 1, :] = qp32[:, :, a]
        q9[:, 3 * a + 2, :] = sq

    na_dev = na64.astype(np.float32)
    dvals = np.einsum('bvc,vc->bv', qp64, na64).astype(np.float32)  # (B, vp)
    if MM2_F32R:
        na_dev = _round_f32r(na_dev)
        dvals = _round_f32r(dvals)
    dna = np.zeros((total_blocks, vp // 128, 128, 4), np.float32)
    dna[:, :, :, 0] = dvals.reshape(total_blocks, vp // 128, 128)
    dna[:, :, :, 1:4] = np.broadcast_to(
        na_dev.reshape(1, vp // 128, 128, 3), (total_blocks, vp // 128, 128, 3))
    dna = np.ascontiguousarray(dna.transpose(2, 0, 1, 3))  # (128, B, vt_n, 4)

    # near-field host correction + min pair distance (one blocked scan)
    slot_of_pt = np.empty(n, np.int64)
    slot_of_pt[slot_orig[valid]] = np.nonzero(valid)[0]
    sqrt_bias0 = float(_MM1_F32R_SQRT_BIAS) if MM1_F32R else 0.0
    min_r2, w_corr = _near_field(
        pts, verts, na, EPS_DEV, R_CUT,
        na_dev[:v].astype(np.float64), dvals[:, :v].astype(np.float64),
        blk_of_pt, o64, p9r=p9, q9r=q9, slot_of_pt=slot_of_pt,
        sqrt_bias=sqrt_bias0)
    if MM1_F32R:
        sqrt_bias = sqrt_bias0
    else:
        sqrt_bias = 0.0 if min_r2 >= 4.0 * _FP32_R2_ERR else float(2.0 * _FP32_R2_ERR)

    key = (npc, vp, sqrt_bias, MM2_F32R, MM1_F32R, EPS_DEV)
    if key not in _PROGRAM_CACHE:
        _PROGRAM_CACHE[key] = _build_program(npc, vp, sqrt_bias, EPS_DEV)
    nc = _PROGRAM_CACHE[key]

    in_maps = []
    for core in range(N_CORES):
        bsl = slice(core * c_n, (core + 1) * c_n)
        in_maps.append({
            "q9": np.ascontiguousarray(q9[bsl].transpose(1, 0, 2)),
            "dna": np.ascontiguousarray(dna[:, bsl]),
            "p9": np.ascontiguousarray(p9[:, core * npc:(core + 1) * npc]),
        })
    try:
        res = run_bass_kernel_spmd(nc, in_maps, core_ids=list(range(N_CORES)))
    except ModuleNotFoundError:
        # BASS_TRACE requested but the axon NTFF hook is unavailable in this
        # container -- rerun untraced.
        import os
        os.environ["BASS_NEVER_TRACE"] = "1"
        res = run_bass_kernel_spmd(nc, in_maps, core_ids=list(range(N_CORES)))
    LAST_RESULTS = res

    dm = np.concatenate([r["dm"] for r in res.results], axis=1)  # (4, ntot)
    dm64 = dm.astype(np.float64)
    w_slot = dm64[0] - (pp64s[:, 0] * dm64[1] + pp64s[:, 1] * dm64[2]
                        + pp64s[:, 2] * dm64[3])
    w = np.empty(n, np.float64)
    w[slot_orig[valid]] = w_slot[valid]
    return w / (4.0 * np.pi) + w_corr


def kernel(verts, faces, output_resolution):
    verts = np.asarray(verts, np.float32)
    faces = np.asarray(faces)
    R = int(output_resolution)
    coords, s, inv, bidx = _grid_and_layout(verts, R)
    na = _normals_areaic(verts, faces)
    pts = coords.reshape(-1, 3)
    w = _winding_device(pts, verts, na, s)
    occ = _sigmoid100(w)
    return _assemble(occ, s, inv, bidx, R)
